# revision 1
# baseline (speedup 1.0000x reference)
"""Trainium2 Bass kernel for nn_Autoregression (16-state AR whitening log-prob).

Math: reference computes log_prob[b,k,t] = -0.5*(C*log(2pi) + logdet(Sigma_k)
+ es_k(t)^T Sigma_k^{-1} es_k(t)) with es = causal_conv(x, W, b).  Since
Sigma^{-1} = L^{-T} L^{-1} and es is affine in x, fold L^{-1} into the conv:
W2 = L^{-1} W, b2 = L^{-1} b, then mahalanobis = sum_c conv(x; W2, b2)^2.

Device layout (per core, T sharded 8 ways with an 8-sample left halo):
conv as matmuls over 128-t chunks producing PSUM [128 t, 512 (8 states x 64
ch)] x 2 halves; contraction packed as (c_in x 2 time-shifts)=128 rows per
step, 4 steps + a 65-row step for the j=8 tap whose ones-row carries the
bias.  ACT squares PSUM -> bf16 SBUF; DVE does the per-state segmented
reduce [128, 8, 64] -> [128, 8]; a small PE transpose flips [128 t, 16 k]
-> [16 k, 128 t]; DVE applies -0.5 and the per-state constant; DMA out.
"""

import os

import numpy as np
import ml_dtypes

import concourse.bass as bass
import concourse.bacc as bacc_mod
import concourse.mybir as mybir
import concourse.tile as tile
from concourse.bass_utils import run_bass_kernel_spmd
from concourse.tile_rust import add_dep_helper

K = 16          # states
C = 64          # channels
T = 65536       # time
AR = 8          # ar order (kernel size AR+1)
NCORES = 8
TLOC = T // NCORES          # 8192 outputs per core
TC = 128                    # outputs per chunk (matmul M)
WAVE = 16                   # chunks per wave (input tile granularity)
WCOLS = TC * WAVE           # 2048 outputs per wave
NW = TLOC // WCOLS          # waves per core
KP = K // 2
NSTEP = 5                   # contraction steps: 4 full + 1 (j=8 + bias row)
NH = 2                      # psum halves (states 0-7, 8-15)

MM_DT = mybir.dt.bfloat16   # conv matmul dtype
SQ_DT = mybir.dt.bfloat16   # squares dtype

_MM_NP = mybir.dt.np(MM_DT)

_CACHE: dict = {}


def _build_program():
    nc = bacc_mod.Bacc()
    f32 = mybir.dt.float32

    # xin rows 0-63: x slice (with halo); rows 64-127: same shifted left by 1
    # (host-duplicated so each wave's xd tile loads with a single DMA)
    xin = nc.declare_dram_parameter("xin", [128, TLOC + AR], MM_DT, isOutput=False)
    # weights as the matmul moving operand: [contraction, step, (half, state, ch)]
    wts = nc.declare_dram_parameter("wts", [128, NSTEP, 1024], MM_DT, isOutput=False)
    ident = nc.declare_dram_parameter("ident", [128, 128], mybir.dt.float32r, isOutput=False)
    biasc = nc.declare_dram_parameter("biasc", [K, 1], f32, isOutput=False)
    onesd = nc.declare_dram_parameter("onesd", [1, WCOLS], MM_DT, isOutput=False)
    out = nc.declare_dram_parameter("out", [K, TLOC], f32, isOutput=True)

    XDW = WCOLS + AR

    with tile.TileContext(nc) as tc:
        with (
            tc.tile_pool(name="singles", bufs=1) as singles,
            # one slot per wave: input DMAs never wait (no slot WAR/WAW)
            tc.tile_pool(name="xpool", bufs=NW) as xpool,
            tc.tile_pool(name="sqpool", bufs=12) as sqpool,
            tc.tile_pool(name="mpool", bufs=6) as mpool,
            tc.tile_pool(name="conv_ps", bufs=5, space="PSUM") as conv_ps,
            tc.tile_pool(name="mt_ps", bufs=2, space="PSUM") as mt_ps,
            tc.tile_pool(name="obs_ps", bufs=1, space="PSUM") as obs_ps,
        ):
            # Matmuls must never be the first PE instruction to observe more
            # than one producer semaphore (1-wait ISA slots; bacc's event-sem
            # legalization costs sequencer time).  pe_observe() emits a tiny
            # 2x2 "reader" matmul whose operands come from a single
            # producer's tile; ordering edges pin readers ahead of the next
            # real matmul.
            scratch = obs_ps.tile([2, 128], f32)
            scratch2 = singles.tile([2, 128], SQ_DT)
            nc.vector.memset(scratch2, 0.0)
            pending = []
            obs_after = [None]

            def pe_observe(col):
                i = nc.tensor.matmul(
                    scratch[0:2, 0:2], col, col, start=True, stop=True
                )
                if obs_after[0] is not None:
                    # not earlier than late in the previous wave, or the PE
                    # FIFO head-of-line blocks on a DMA that hasn't landed
                    add_dep_helper(i.ins, obs_after[0].ins, sync=False)
                pending.append(i)

            def _flush(i):
                while pending:
                    add_dep_helper(i.ins, pending.pop().ins, sync=False)
                return i

            def pe_matmul(*args, **kw):
                return _flush(nc.tensor.matmul(*args, **kw))

            # dep-free warmup matmuls: keep the PE busy through the initial
            # input DMAs so HAM un-throttles before real work (N=128 streams
            # so the activity monitor sees a busy array)
            for _ in range(35):
                nc.tensor.matmul(
                    scratch[0:2, 0:128],
                    scratch2[0:2, 0:2],
                    scratch2[0:2, 0:128],
                    start=True,
                    stop=True,
                )

            # DMA issue plan: sync HWDGE ring carries the critical path
            # (first xd half, per-step weights, second xd half);
            # prefetchables (identity, bias, xe, waves 1-3) go on the scalar
            # engine's separate ring.
            w_sb = singles.tile([128, NSTEP, 1024], MM_DT)
            ident_sb = singles.tile([128, 128], mybir.dt.float32r)
            bias_sb = singles.tile([K, 1], f32)
            out_sb = singles.tile([K, TLOC], f32)
            xds, xes = [], []
            sc_dmas = []
            sc_dmas.append(nc.scalar.dma_start(out=ident_sb, in_=ident[:, :]))
            sc_dmas.append(nc.scalar.dma_start(out=bias_sb, in_=biasc[:, :]))
            for w in range(NW):
                base = w * WCOLS
                # xd: rows 0-63 = xin shifts (j even), rows 64-127 = xin
                # shifted one further (j odd).  xe: rows 0-63 = xin shift 8,
                # row 64 = ones (bias row).
                xd = xpool.tile([128, XDW], MM_DT, name="xd")
                xe = xpool.tile([C + 1, WCOLS], MM_DT, name="xe")
                if w == 0:
                    nc.sync.dma_start(out=xd[:, 0:1036], in_=xin[:, 0:1036])
                    for s in range(NSTEP):
                        nc.sync.dma_start(
                            out=w_sb[:, s, :], in_=wts[:, s, :]
                        )
                    nc.sync.dma_start(out=xd[:, 1036:XDW], in_=xin[:, 1036:XDW])
                    sc_dmas.append(
                        nc.scalar.dma_start(
                            out=xe[0:C, :],
                            in_=xin[0:C, base + AR : base + AR + WCOLS],
                        )
                    )
                    sc_dmas.append(
                        nc.scalar.dma_start(out=xe[C : C + 1, :], in_=onesd[:, :])
                    )
                elif w == 1:
                    sc_dmas.append(
                        nc.scalar.dma_start(out=xd, in_=xin[:, base : base + XDW])
                    )
                    sc_dmas.append(
                        nc.scalar.dma_start(
                            out=xe[0:C, :],
                            in_=xin[0:C, base + AR : base + AR + WCOLS],
                        )
                    )
                    sc_dmas.append(
                        nc.scalar.dma_start(out=xe[C : C + 1, :], in_=onesd[:, :])
                    )
                xds.append(xd)
                xes.append(xe)

            def load_wave_inputs(w):
                # waves 2-3 load lazily (two waves ahead) so the prefetch
                # doesn't flood the DMA fabric while wave 0 computes
                base = w * WCOLS
                nc.scalar.dma_start(out=xds[w], in_=xin[:, base : base + XDW])
                nc.scalar.dma_start(
                    out=xes[w][0:C, :],
                    in_=xin[0:C, base + AR : base + AR + WCOLS],
                )
                nc.scalar.dma_start(out=xes[w][C : C + 1, :], in_=onesd[:, :])
            # DVE observer for the bias DMA (TS struct fits one wait)
            dve_scratch = singles.tile([K, 1], f32)
            nc.vector.tensor_copy(dve_scratch, bias_sb)

            first_sq = [True]

            def chunk_tail(w, off, psh):
                base = w * WCOLS
                m_sb = mpool.tile([128, K], mybir.dt.float32r, name="m_sb")
                for h in range(NH):
                    sq = sqpool.tile([128, 512], SQ_DT, name="sq", tag="sq")
                    sq_i = nc.scalar.activation(
                        sq, psh[h], mybir.ActivationFunctionType.Square
                    )
                    if first_sq[0]:
                        # the Act sequencer must issue every prefetch DMA
                        # before its first square, else a square that
                        # transitively gates one of those DMAs deadlocks
                        while sc_dmas:
                            add_dep_helper(sq_i.ins, sc_dmas.pop().ins, sync=False)
                        first_sq[0] = False
                    with nc.allow_low_precision(
                        reason="float32r shares float32 bits; r-mode only "
                        "affects the PE multiply path"
                    ):
                        nc.vector.tensor_reduce(
                            out=m_sb[:, 8 * h : 8 * h + 8],
                            in_=sq.rearrange("p (g c) -> p g c", g=8),
                            axis=mybir.AxisListType.X,
                            op=mybir.AluOpType.add,
                        )
                mt = mt_ps.tile([K, TC], mybir.dt.float32r, name="mt")
                _flush(nc.tensor.transpose(mt, m_sb, ident_sb))
                nc.vector.tensor_scalar(
                    out=out_sb[:, base + off : base + off + TC],
                    in0=mt[0:K, :],
                    scalar1=-0.5,
                    scalar2=bias_sb,
                    op0=mybir.AluOpType.mult,
                    op1=mybir.AluOpType.add,
                )

            def conv_lhsT(xd, xe, off, s):
                if s < 4:
                    return xd[:, off + 2 * s : off + 2 * s + TC]
                return xe[0 : C + 1, off : off + TC]

            def conv_rhs(s, h):
                if s < 4:
                    return w_sb[:, s, 512 * h : 512 * h + 512]
                return w_sb[0 : C + 1, s, 512 * h : 512 * h + 512]

            for w in range(NW):
                base = w * WCOLS
                xd = xds[w]
                xe = xes[w]
                if w + 2 < NW:
                    load_wave_inputs(w + 2)
                if w == 0:
                    # first four chunks pairwise s-major: the weight steps
                    # arrive one-by-one on the sync ring, so advance both
                    # chunks per step instead of stalling per chunk
                    for pair in ((0, 1), (2, 3)):
                        pshp = {
                            (c, h): conv_ps.tile(
                                [128, 512], f32, name=f"ps{c}{h}", tag="ps"
                            )
                            for c in pair
                            for h in range(NH)
                        }
                        for s in range(NSTEP):
                            if pair[0] == 0 and s == 0:
                                pe_observe(ident_sb[:, 0:2])
                                pe_observe(xd[:, 0:2])
                            if pair[0] == 0 and s == 4:
                                pe_observe(xe[0:C, 0:2])
                                pe_observe(xe[C : C + 1, 0:2])
                            for c in pair:
                                lhsT = conv_lhsT(xd, xe, c * TC, s)
                                for h in range(NH):
                                    pe_matmul(
                                        pshp[c, h],
                                        lhsT,
                                        conv_rhs(s, h),
                                        start=(s == 0),
                                        stop=(s == 4),
                                    )
                        for c in pair:
                            chunk_tail(w, c * TC, [pshp[c, h] for h in range(NH)])
                    start_tcl = 4
                else:
                    start_tcl = 0
                for tcl in range(start_tcl, WAVE):
                    off = tcl * TC
                    psh = [
                        conv_ps.tile([128, 512], f32, name=f"ps{h}", tag="ps")
                        for h in range(NH)
                    ]
                    for s in range(NSTEP):
                        if w > 0 and tcl == 0 and s == 0:
                            pe_observe(xd[:, 0:2])
                        if w > 0 and tcl == 0 and s == 4:
                            # lazily: s0-s3 must not stall on the xe loads
                            pe_observe(xe[0:C, 0:2])
                            pe_observe(xe[C : C + 1, 0:2])
                        lhsT = conv_lhsT(xd, xe, off, s)
                        for h in range(NH):
                            mm_i = pe_matmul(
                                psh[h],
                                lhsT,
                                conv_rhs(s, h),
                                start=(s == 0),
                                stop=(s == 4),
                            )
                    if tcl == WAVE - 2:
                        obs_after[0] = mm_i
                    chunk_tail(w, off, psh)
                if w < NW - 1:
                    nc.sync.dma_start(
                        out=out[:, base : base + WCOLS],
                        in_=out_sb[:, base : base + WCOLS],
                    )
                else:
                    # last wave: quarter DMAs so the final store is tiny
                    for q in range(4):
                        uq = base + q * (WCOLS // 4)
                        nc.sync.dma_start(
                            out=out[:, uq : uq + WCOLS // 4],
                            in_=out_sb[:, uq : uq + WCOLS // 4],
                        )
    nc.compile()
    return nc


def _prep_host(W, b, Sigma):
    """Fold L^{-1} into conv weights; pack moving-operand tiles, constants."""
    W64 = W.astype(np.float64)
    b64 = b.astype(np.float64)
    S64 = Sigma.astype(np.float64)
    L = np.linalg.cholesky(S64)
    Li = np.linalg.inv(L)                       # [K, C, C] lower-triangular inv
    logdet = 2.0 * np.sum(np.log(np.diagonal(L, axis1=1, axis2=2)), axis=1)
    W2 = np.einsum("kdc,kcij->kdij", Li, W64)   # [K, C(d), C(ci), 9]
    b2 = np.einsum("kdc,kc->kd", Li, b64)       # [K, C]

    # moving operand: w_np[r, s, 512*(k//8) + 64*(k%8) + d]
    #   s<4:  r = ci + 64*joff -> W2[k, d, ci, 2s+joff]
    #   s==4: r<64 -> W2[k, d, r, 8]; r==64 -> b2[k, d]; else 0
    w_np = np.zeros((128, NSTEP, 1024), np.float32)
    for s in range(4):
        # [ci + 64*joff, (k, d)]
        sub = W2[:, :, :, 2 * s : 2 * s + 2]        # [k, d, ci, joff]
        w_np[:, s, :] = np.transpose(sub, (3, 2, 0, 1)).reshape(128, 1024)
    w_np[0:C, 4, :] = np.transpose(W2[:, :, :, 8], (2, 0, 1)).reshape(C, 1024)
    w_np[C, 4, :] = b2.reshape(1024)

    const = C * np.log(2.0 * np.pi) + logdet
    bias_np = (-0.5 * const).astype(np.float32).reshape(K, 1)
    return w_np, bias_np


def _run(x, W, b, Sigma, trace=False):
    x = np.asarray(x, np.float32)
    W = np.asarray(W, np.float32)
    b = np.asarray(b, np.float32)
    Sigma = np.asarray(Sigma, np.float32)
    if "nc" not in _CACHE:
        _CACHE["nc"] = _build_program()
    nc = _CACHE["nc"]
    w_np, bias_np = _prep_host(W, b, Sigma)

    # left causal pad (AR) plus one right pad col so the shifted copy of the
    # last core's slice stays in bounds
    xpad = np.pad(np.asarray(x, np.float32)[0], ((0, 0), (AR, 1)))  # [C, T+9]
    in_maps = []
    for i in range(NCORES):
        lo = xpad[:, TLOC * i : TLOC * i + TLOC + AR]
        hi = xpad[:, TLOC * i + 1 : TLOC * i + TLOC + AR + 1]
        in_maps.append(
            {
                "xin": np.ascontiguousarray(
                    np.concatenate([lo, hi], axis=0).astype(_MM_NP)
                ),
                "wts": w_np.astype(_MM_NP),
                "ident": np.eye(128, dtype=np.float32),
                "biasc": bias_np,
                "onesd": np.ones((1, WCOLS), _MM_NP),
            }
        )
    res = run_bass_kernel_spmd(
        nc, in_maps, core_ids=list(range(NCORES)), trace=trace
    )
    outs = [res.results[i]["out"] for i in range(NCORES)]
    full = np.concatenate(outs, axis=1)[None]   # [1, K, T]
    return full.astype(np.float32), res


def kernel(x, W, b, Sigma):
    out, _ = _run(x, W, b, Sigma, trace=bool(int(os.environ.get("BASS_TRACE", "0"))))
    return out



# revision 2
# speedup vs baseline: 1.6465x; 1.6465x over previous
"""Trainium2 Bass kernel for nn_Autoregression (16-state AR whitening log-prob).

Math: reference computes log_prob[b,k,t] = -0.5*(C*log(2pi) + logdet(Sigma_k)
+ es_k(t)^T Sigma_k^{-1} es_k(t)) with es = causal_conv(x, W, b).  Since
Sigma^{-1} = L^{-T} L^{-1} and es is affine in x, fold L^{-1} into the conv:
W2 = L^{-1} W, b2 = L^{-1} b, then mahalanobis = sum_c conv(x; W2, b2)^2.

fp8 DoubleRow layout (per core, T sharded 8 ways):
taps 1-8 of the 9-tap conv run as fp8e4 DoubleRow matmuls -- the PE array
virtualizes to 256 contraction rows, so 2 DR matmuls replace 4 bf16 ones.
Weights carry a per-state scale s_k (max |W2_k| -> 96) so e4m3 quantization
error stays relative; the scale divides out in the final tensor_scalar
(per-partition scalar1 = -0.5/s_k^2).  Tap 0 + the bias row stay bf16
(1 matmul, 65 contraction rows).  Per 128-t chunk: 4 DR + 2 bf16 matmuls
into PSUM [128 t, 512 (8 states x 64 ch)] x 2 halves.  ACT squares PSUM ->
bf16 SBUF; DVE does the segmented reduce [128, 8, 64] -> [128, 8] into a
[128, 128] tile covering 8 chunks; one PE transpose per 8 chunks flips
[128 t-rows, (chunk, state)] -> [(chunk, state), 128 t]; DVE applies the
per-state scale and constant; DMA out (host de-interleaves chunk rows).
"""

import os

import numpy as np
import ml_dtypes

import concourse.bass as bass
import concourse.bacc as bacc_mod
import concourse.mybir as mybir
import concourse.tile as tile
from concourse.bass_utils import run_bass_kernel_spmd
from concourse.tile_rust import add_dep_helper

K = 16          # states
C = 64          # channels
T = 65536       # time
AR = 8          # ar order (kernel size AR+1)
NCORES = 8
TLOC = T // NCORES          # 8192 outputs per core
TC = 128                    # outputs per chunk (matmul M)
WAVE = 16                   # chunks per wave (input tile granularity)
WCOLS = TC * WAVE           # 2048 outputs per wave
NW = TLOC // WCOLS          # waves per core
NH = 2                      # psum halves (states 0-7, 8-15)
NPAIR = 2                   # DoubleRow matmuls per half (taps 1-4, 5-8)
GRP = 8                     # chunks per transpose group

FP8_DT = mybir.dt.float8e4
TAIL_DT = mybir.dt.bfloat16  # tap-0 + bias matmul dtype
SQ_DT = mybir.dt.bfloat16    # squares dtype

_FP8_NP = mybir.dt.np(FP8_DT)
_TAIL_NP = mybir.dt.np(TAIL_DT)

DR = mybir.MatmulPerfMode.DoubleRow

_CACHE: dict = {}


def _build_program():
    nc = bacc_mod.Bacc()
    f32 = mybir.dt.float32

    # xq plane p (p=0..3): rows 0-63 = x shifted by 1+2p, rows 64-127 = x
    # shifted by 2+2p (host-duplicated); DR pair P reads planes 2P, 2P+1.
    xq = nc.declare_dram_parameter("xq", [128, 4, TLOC], FP8_DT, isOutput=False)
    # tap-0 x (rows 0-63) + ones row (bias) in bf16
    xeb = nc.declare_dram_parameter("xeb", [C + 1, TLOC], TAIL_DT, isOutput=False)
    # fp8 weights: [contraction row, pair, plane, (half, state, ch)]
    wq = nc.declare_dram_parameter("wq", [128, NPAIR, 2, 1024], FP8_DT, isOutput=False)
    # bf16 tail weights: rows 0-63 tap-0, row 64 bias
    w8 = nc.declare_dram_parameter("w8", [C + 1, 1024], TAIL_DT, isOutput=False)
    ident = nc.declare_dram_parameter("ident", [128, 128], mybir.dt.float32r, isOutput=False)
    # per-row (16c+k) output affine: col 0 = -0.5/s_k^2, col 1 = -0.5*const_k
    vecs = nc.declare_dram_parameter("vecs", [128, 2], f32, isOutput=False)
    # out[w, 16c'+k, g, m] = log_prob[k, w*2048 + g*1024 + c'*128 + m]
    out = nc.declare_dram_parameter("out", [NW, 128, NH, TC], f32, isOutput=True)

    with tile.TileContext(nc) as tc:
        with (
            tc.tile_pool(name="singles", bufs=1) as singles,
            # one slot per wave: input DMAs never wait (no slot WAR/WAW)
            tc.tile_pool(name="xpool", bufs=NW) as xpool,
            tc.tile_pool(name="sqpool", bufs=12) as sqpool,
            tc.tile_pool(name="mpool", bufs=4) as mpool,
            tc.tile_pool(name="conv_ps", bufs=5, space="PSUM") as conv_ps,
            tc.tile_pool(name="mt_ps", bufs=2, space="PSUM") as mt_ps,
            tc.tile_pool(name="obs_ps", bufs=1, space="PSUM") as obs_ps,
        ):
            # Matmuls must never be the first PE instruction to observe more
            # than one producer semaphore (1-wait ISA slots; bacc's event-sem
            # legalization costs sequencer time).  pe_observe() emits a tiny
            # 2x2 "reader" matmul whose operands come from a single
            # producer's tile; ordering edges pin readers ahead of the next
            # real matmul.
            scratch = obs_ps.tile([2, 128], f32)
            scratch2 = singles.tile([2, 128], SQ_DT)
            nc.vector.memset(scratch2, 0.0)
            pending = []
            obs_after = [None]

            def pe_observe(col):
                i = nc.tensor.matmul(
                    scratch[0:2, 0:2], col, col, start=True, stop=True
                )
                if obs_after[0] is not None:
                    # not earlier than late in the previous wave, or the PE
                    # FIFO head-of-line blocks on a DMA that hasn't landed
                    add_dep_helper(i.ins, obs_after[0].ins, sync=False)
                pending.append(i)

            def _flush(i):
                while pending:
                    add_dep_helper(i.ins, pending.pop().ins, sync=False)
                return i

            def pe_matmul(*args, **kw):
                return _flush(nc.tensor.matmul(*args, **kw))

            # dep-free warmup matmuls: keep the PE busy through the initial
            # input DMAs so HAM un-throttles before real work
            for _ in range(35):
                nc.tensor.matmul(
                    scratch[0:2, 0:128],
                    scratch2[0:2, 0:2],
                    scratch2[0:2, 0:128],
                    start=True,
                    stop=True,
                )

            # DMA issue plan: sync HWDGE ring carries the critical path
            # (wave-0 inputs piecewise + weights); prefetchables (identity,
            # vecs, wave 1) go on the scalar engine's separate ring.
            wq_sb = singles.tile([128, NPAIR, 2, 1024], FP8_DT)
            w8_sb = singles.tile([C + 1, 1024], TAIL_DT)
            ident_sb = singles.tile([128, 128], mybir.dt.float32r)
            vec_sb = singles.tile([128, 2], f32)
            out_sb = singles.tile([128, NW * NH, TC], f32)
            xqs, xes = [], []
            sc_dmas = []
            sc_dmas.append(nc.scalar.dma_start(out=ident_sb, in_=ident[:, :]))
            sc_dmas.append(nc.scalar.dma_start(out=vec_sb, in_=vecs[:, :]))
            # wave-0 column splits: chunk c unblocks at piece covering c*TC+TC
            W0_CUTS = [0, 136, 648, 1160, 1672, WCOLS]
            for w in range(NW):
                xq_sb = xpool.tile([128, 4, WCOLS], FP8_DT, name="xq")
                xe_sb = xpool.tile([C + 1, WCOLS], TAIL_DT, name="xe")
                base = w * WCOLS
                if w == 0:
                    lo, hi = W0_CUTS[0], W0_CUTS[1]
                    nc.sync.dma_start(
                        out=xq_sb[:, :, lo:hi], in_=xq[:, :, base + lo : base + hi]
                    )
                    nc.sync.dma_start(out=wq_sb[:, 0], in_=wq[:, 0])
                    nc.sync.dma_start(out=wq_sb[:, 1], in_=wq[:, 1])
                    nc.sync.dma_start(out=w8_sb, in_=w8[:, :])
                    nc.sync.dma_start(
                        out=xe_sb[:, lo:hi], in_=xeb[:, base + lo : base + hi]
                    )
                    for ci in range(1, len(W0_CUTS) - 1):
                        lo, hi = W0_CUTS[ci], W0_CUTS[ci + 1]
                        nc.sync.dma_start(
                            out=xq_sb[:, :, lo:hi],
                            in_=xq[:, :, base + lo : base + hi],
                        )
                        nc.sync.dma_start(
                            out=xe_sb[:, lo:hi], in_=xeb[:, base + lo : base + hi]
                        )
                elif w == 1:
                    sc_dmas.append(
                        nc.scalar.dma_start(
                            out=xq_sb, in_=xq[:, :, base : base + WCOLS]
                        )
                    )
                    sc_dmas.append(
                        nc.scalar.dma_start(
                            out=xe_sb, in_=xeb[:, base : base + WCOLS]
                        )
                    )
                xqs.append(xq_sb)
                xes.append(xe_sb)

            def load_wave_inputs(w):
                # waves 2-3 load lazily (two waves ahead) so the prefetch
                # doesn't flood the DMA fabric while wave 0 computes
                base = w * WCOLS
                nc.scalar.dma_start(out=xqs[w], in_=xq[:, :, base : base + WCOLS])
                nc.scalar.dma_start(out=xes[w], in_=xeb[:, base : base + WCOLS])

            # DVE observer for the vecs DMA (TS struct fits one wait)
            dve_scratch = singles.tile([128, 2], f32)
            nc.vector.tensor_copy(dve_scratch, vec_sb)

            first_sq = [True]

            for w in range(NW):
                xq_sb = xqs[w]
                xe_sb = xes[w]
                if w + 2 < NW:
                    load_wave_inputs(w + 2)
                m8 = None
                for c in range(WAVE):
                    off = c * TC
                    g, cp = c // GRP, c % GRP
                    if cp == 0:
                        m8 = mpool.tile([128, 128], mybir.dt.float32r, name="m8")
                    psh = [
                        conv_ps.tile([128, 512], f32, name=f"ps{h}", tag="ps")
                        for h in range(NH)
                    ]
                    if w == 0 and c == 0:
                        pe_observe(xq_sb[:, 0, 0:2])
                        pe_observe(w8_sb[0:2, 0:2])
                    elif c == 0:
                        pe_observe(xq_sb[:, 0, 0:2])
                        pe_observe(xe_sb[0:2, 0:2])
                    for P in range(NPAIR):
                        lhsT = xq_sb[:, 2 * P : 2 * P + 2, off : off + TC]
                        for h in range(NH):
                            pe_matmul(
                                psh[h],
                                lhsT,
                                wq_sb[:, P, :, 512 * h : 512 * h + 512],
                                start=(P == 0),
                                stop=False,
                                perf_mode=DR,
                            )
                    for h in range(NH):
                        mm_i = pe_matmul(
                            psh[h],
                            xe_sb[:, off : off + TC],
                            w8_sb[:, 512 * h : 512 * h + 512],
                            start=False,
                            stop=True,
                        )
                    if c == WAVE - 2:
                        obs_after[0] = mm_i
                    # squares + segmented reduce into the group tile
                    for h in range(NH):
                        sq = sqpool.tile([128, 512], SQ_DT, name="sq", tag="sq")
                        sq_i = nc.scalar.activation(
                            sq, psh[h], mybir.ActivationFunctionType.Square
                        )
                        if first_sq[0]:
                            # the Act sequencer must issue every prefetch DMA
                            # before its first square, else a square that
                            # transitively gates one of those DMAs deadlocks
                            while sc_dmas:
                                add_dep_helper(sq_i.ins, sc_dmas.pop().ins, sync=False)
                            first_sq[0] = False
                        with nc.allow_low_precision(
                            reason="float32r shares float32 bits; r-mode only "
                            "affects the PE multiply path"
                        ):
                            nc.vector.tensor_reduce(
                                out=m8[:, 16 * cp + 8 * h : 16 * cp + 8 * h + 8],
                                in_=sq.rearrange("p (g c) -> p g c", g=8),
                                axis=mybir.AxisListType.X,
                                op=mybir.AluOpType.add,
                            )
                    if cp == GRP - 1:
                        mt = mt_ps.tile([128, 128], mybir.dt.float32r, name="mt")
                        _flush(nc.tensor.transpose(mt, m8, ident_sb))
                        nc.vector.tensor_scalar(
                            out=out_sb[:, NH * w + g, :],
                            in0=mt,
                            scalar1=vec_sb[:, 0:1],
                            scalar2=vec_sb[:, 1:2],
                            op0=mybir.AluOpType.mult,
                            op1=mybir.AluOpType.add,
                        )
                        if w == NW - 1 and g == NH - 1:
                            # last store split so the final DMA is small
                            nc.sync.dma_start(
                                out=out[w, 0:64, g, :],
                                in_=out_sb[0:64, NH * w + g, :],
                            )
                            nc.sync.dma_start(
                                out=out[w, 64:128, g, :],
                                in_=out_sb[64:128, NH * w + g, :],
                            )
                        else:
                            nc.sync.dma_start(
                                out=out[w, :, g, :], in_=out_sb[:, NH * w + g, :]
                            )
    nc.compile()
    return nc


def _prep_host(W, b, Sigma):
    """Fold L^{-1} + per-state fp8 scale into conv weights; pack tiles."""
    W64 = W.astype(np.float64)
    b64 = b.astype(np.float64)
    S64 = Sigma.astype(np.float64)
    L = np.linalg.cholesky(S64)
    Li = np.linalg.inv(L)                       # [K, C, C] lower-triangular inv
    logdet = 2.0 * np.sum(np.log(np.diagonal(L, axis1=1, axis2=2)), axis=1)
    W2 = np.einsum("kdc,kcij->kdij", Li, W64)   # [K, C(d), C(ci), 9]
    b2 = np.einsum("kdc,kc->kd", Li, b64)       # [K, C]

    sk = 96.0 / np.abs(W2).max(axis=(1, 2, 3))  # per-state fp8 range scale
    W2s = (W2 * sk[:, None, None, None]).astype(np.float32)
    b2s = (b2 * sk[:, None]).astype(np.float32)

    def kd_cols(a):  # [K, C(d), C(ci)] -> [C(ci), 1024] with col = 64k + d
        return np.ascontiguousarray(np.transpose(a, (2, 0, 1)).reshape(C, 1024))

    wq_np = np.zeros((128, NPAIR, 2, 1024), np.float32)
    for P in range(NPAIR):
        for i in range(2):
            j = 1 + 4 * P + 2 * i
            wq_np[0:C, P, i, :] = kd_cols(W2s[:, :, :, j])
            wq_np[C:128, P, i, :] = kd_cols(W2s[:, :, :, j + 1])
    w8_np = np.zeros((C + 1, 1024), np.float32)
    w8_np[0:C, :] = kd_cols(W2s[:, :, :, 0])
    w8_np[C, :] = b2s.reshape(1024)

    const = C * np.log(2.0 * np.pi) + logdet
    vecs_np = np.empty((128, 2), np.float32)
    kk = np.arange(128) % K
    vecs_np[:, 0] = -0.5 / (sk[kk] ** 2)
    vecs_np[:, 1] = -0.5 * const[kk]
    return wq_np.astype(_FP8_NP), w8_np.astype(_TAIL_NP), vecs_np


def _run(x, W, b, Sigma, trace=False):
    x = np.asarray(x, np.float32)
    W = np.asarray(W, np.float32)
    b = np.asarray(b, np.float32)
    Sigma = np.asarray(Sigma, np.float32)
    if "nc" not in _CACHE:
        _CACHE["nc"] = _build_program()
    nc = _CACHE["nc"]
    wq_np, w8_np, vecs_np = _prep_host(W, b, Sigma)

    xpad = np.pad(x[0], ((0, 0), (AR, 0)))      # [C, T+8] left causal pad
    x8 = xpad.astype(_FP8_NP)                   # quantize once
    xb = xpad.astype(_TAIL_NP)
    ident_np = np.eye(128, dtype=np.float32)
    in_maps = []
    for i in range(NCORES):
        o = TLOC * i
        xq_np = np.empty((128, 4, TLOC), _FP8_NP)
        for p in range(4):
            xq_np[0:C, p, :] = x8[:, o + 1 + 2 * p : o + 1 + 2 * p + TLOC]
            xq_np[C:128, p, :] = x8[:, o + 2 + 2 * p : o + 2 + 2 * p + TLOC]
        xe_np = np.empty((C + 1, TLOC), _TAIL_NP)
        xe_np[0:C, :] = xb[:, o : o + TLOC]
        xe_np[C, :] = np.ones(TLOC, _TAIL_NP)
        in_maps.append(
            {
                "xq": xq_np,
                "xeb": xe_np,
                "wq": wq_np,
                "w8": w8_np,
                "ident": ident_np,
                "vecs": vecs_np,
            }
        )
    res = run_bass_kernel_spmd(
        nc, in_maps, core_ids=list(range(NCORES)), trace=trace
    )
    outs = []
    for i in range(NCORES):
        o = res.results[i]["out"]               # [NW, 128, NH, TC]
        o = o.reshape(NW, GRP, K, NH, TC)       # rows -> (c', k)
        o = np.transpose(o, (2, 0, 3, 1, 4)).reshape(K, TLOC)
        outs.append(o)
    full = np.concatenate(outs, axis=1)[None]   # [1, K, T]
    return np.ascontiguousarray(full.astype(np.float32)), res


def kernel(x, W, b, Sigma):
    out, _ = _run(x, W, b, Sigma, trace=bool(int(os.environ.get("BASS_TRACE", "0"))))
    return out
